# revision 1
# baseline (speedup 1.0000x reference)
"""Trainium2 Bass kernel for nn_DPB_24026047054525 (dense_mlp, memory-bound).

Computes: out[b,h,i,j] = mask[b,h,i,j] - bias[h,i,j] where
bias[h,i,j] = MLP((i-j)/DIVIDE)[h], MLP = 3x(Linear+SiLU) + Linear, widths
1->64->64->64->8.

Key structural insight: rel = (i-j) takes only 2047 distinct values, so the
MLP runs on a tiny per-core table of 1152 scalars instead of the 1M-point
grid.  The (128-row, 1024-col) per-core bias tile is a Toeplitz expansion of
that table:

    bias[p, j] = T[127 - p + j],   T[s] = MLP((r0 + 127 - s)/DIVIDE)

which we materialize on-chip as (a) a positive-stride overlapping "Hankel"
DMA  H[k, h, t] = T[h, k + t]  from a DRAM scratch copy of the table, and
(b) a partition-order reversal via a PE matmul against the anti-identity
matrix J (out[p] = H[127-p]).  All DMAs are vanilla (partition-dim-first,
positive strides).

Sharding: L1 (rows of the rel grid) split across 8 cores, 128 rows each;
mask/out sharded the same way; MLP weights replicated; subtract local.
q/k/v are unused by the reference computation and are never transferred.
"""

import os
import sys

import numpy as np

for _p in ("/opt/trn_rl_repo",):
    if _p not in sys.path and os.path.isdir(_p):
        sys.path.append(_p)

import concourse.bacc as bacc
import concourse.mybir as mybir
import concourse.tile as tile
from concourse.ap import AP
from concourse.bass_utils import run_bass_kernel_spmd

B = 2
NH = 8
L = 1024
DIM = 64
NCORES = 8
RPC = L // NCORES  # rows per core = 128
T = 1152  # per-core rel-table length (covers s in [0, 127+1023])
DIVIDE = 1.0

F32 = mybir.dt.float32

_CACHE = {}

# Filled by kernel() on every call: the BassKernelResults of the last run
# (results, exec_time_ns when tracing is available, ...).
LAST_RESULTS = None


def _build_program():
    nc = bacc.Bacc("TRN2", target_bir_lowering=False, debug=False)

    mask_sh = nc.dram_tensor("mask_shard", [B, NH, RPC, L], F32, kind="ExternalInput")
    relrow = nc.dram_tensor("relrow", [1, T], F32, kind="ExternalInput")
    w0 = nc.dram_tensor("w0", [1, DIM], F32, kind="ExternalInput")
    w1 = nc.dram_tensor("w1", [DIM, DIM], F32, kind="ExternalInput")
    w2 = nc.dram_tensor("w2", [DIM, DIM], F32, kind="ExternalInput")
    wf = nc.dram_tensor("wf", [DIM, NH], F32, kind="ExternalInput")
    b0 = nc.dram_tensor("b0c", [DIM, 1], F32, kind="ExternalInput")
    b1 = nc.dram_tensor("b1c", [DIM, 1], F32, kind="ExternalInput")
    b2 = nc.dram_tensor("b2c", [DIM, 1], F32, kind="ExternalInput")
    bf = nc.dram_tensor("bfc", [NH, 1], F32, kind="ExternalInput")
    jrev = nc.dram_tensor("jrev", [128, 128], F32, kind="ExternalInput")
    out_sh = nc.dram_tensor("out_shard", [B, NH, RPC, L], F32, kind="ExternalOutput")

    tbl_dram = nc.dram_tensor("tbl_scratch", [NH, T], F32)

    with tile.TileContext(nc) as tc:
        with (
            tc.tile_pool(name="consts", bufs=1) as consts,
            tc.tile_pool(name="mlp", bufs=2) as mlp,
            tc.tile_pool(name="mlp_ps", bufs=2, space="PSUM") as mlp_ps,
            tc.tile_pool(name="rev_ps", bufs=4, space="PSUM") as rev_ps,
            tc.tile_pool(name="bias", bufs=1) as biasp,
            tc.tile_pool(name="hank", bufs=1) as hankp,
            tc.tile_pool(name="io", bufs=6) as io,
        ):
            # ---- constants to SBUF (tiny) ----
            w0s = consts.tile([1, DIM], F32)
            nc.gpsimd.dma_start(out=w0s[:], in_=w0.ap())
            w1s = consts.tile([DIM, DIM], F32)
            nc.gpsimd.dma_start(out=w1s[:], in_=w1.ap())
            w2s = consts.tile([DIM, DIM], F32)
            nc.gpsimd.dma_start(out=w2s[:], in_=w2.ap())
            wfs = consts.tile([DIM, NH], F32)
            nc.gpsimd.dma_start(out=wfs[:], in_=wf.ap())
            b0s = consts.tile([DIM, 1], F32)
            nc.gpsimd.dma_start(out=b0s[:], in_=b0.ap())
            b1s = consts.tile([DIM, 1], F32)
            nc.gpsimd.dma_start(out=b1s[:], in_=b1.ap())
            b2s = consts.tile([DIM, 1], F32)
            nc.gpsimd.dma_start(out=b2s[:], in_=b2.ap())
            bfs = consts.tile([NH, 1], F32)
            nc.gpsimd.dma_start(out=bfs[:], in_=bf.ap())
            jrevs = consts.tile([128, 128], F32)
            nc.gpsimd.dma_start(out=jrevs[:], in_=jrev.ap())
            rels = consts.tile([1, T], F32)
            nc.gpsimd.dma_start(out=rels[:], in_=relrow.ap())

            # ---- MLP over the T-token rel table, transposed layout ----
            # h*T[d, tok]; final table g[NH, T] with heads on partitions.
            gsb = mlp.tile([NH, T], F32, tag="gtable")
            silu = mybir.ActivationFunctionType.Silu
            ident = mybir.ActivationFunctionType.Identity
            for lo, hi in ((0, 512), (512, 1024), (1024, T)):
                n = hi - lo
                ps0 = mlp_ps.tile([DIM, n], F32, tag="ps")
                nc.tensor.matmul(
                    ps0[:], lhsT=w0s[:], rhs=rels[:, lo:hi], start=True, stop=True
                )
                h0 = mlp.tile([DIM, n], F32, tag="h")
                nc.scalar.activation(h0[:], ps0[:], silu, bias=b0s[:, 0:1], scale=1.0)

                ps1 = mlp_ps.tile([DIM, n], F32, tag="ps")
                nc.tensor.matmul(ps1[:], lhsT=w1s[:], rhs=h0[:], start=True, stop=True)
                h1 = mlp.tile([DIM, n], F32, tag="h")
                nc.scalar.activation(h1[:], ps1[:], silu, bias=b1s[:, 0:1], scale=1.0)

                ps2 = mlp_ps.tile([DIM, n], F32, tag="ps")
                nc.tensor.matmul(ps2[:], lhsT=w2s[:], rhs=h1[:], start=True, stop=True)
                h2 = mlp.tile([DIM, n], F32, tag="h")
                nc.scalar.activation(h2[:], ps2[:], silu, bias=b2s[:, 0:1], scale=1.0)

                psf = mlp_ps.tile([NH, n], F32, tag="psf")
                nc.tensor.matmul(psf[:], lhsT=wfs[:], rhs=h2[:], start=True, stop=True)
                nc.scalar.activation(
                    gsb[:, lo:hi], psf[:], ident, bias=bfs[:, 0:1], scale=1.0
                )

            # ---- table to DRAM scratch, then Hankel slab back to SBUF ----
            nc.gpsimd.dma_start(out=tbl_dram.ap(), in_=gsb[:])
            # H[k, h, t] = T[h, k + t]   (overlapping reads, positive strides)
            hank = hankp.tile([128, NH * L], F32)
            tbl_h = tbl_dram.ap().tensor
            for h in range(NH):
                src = AP(tbl_h, h * T, [[1, 128], [1, L]])
                nc.gpsimd.dma_start(out=hank[:, h * L : (h + 1) * L], in_=src)

            # ---- partition reversal via PE: bias[p] = H[127 - p] ----
            bias_sb = biasp.tile([128, NH * L], F32)
            for h in range(NH):
                for c in range(2):
                    lo = h * L + c * 512
                    pb = rev_ps.tile([128, 512], F32, tag="rev")
                    nc.tensor.matmul(
                        pb[:], lhsT=jrevs[:], rhs=hank[:, lo : lo + 512],
                        start=True, stop=True,
                    )
                    nc.vector.tensor_copy(bias_sb[:, lo : lo + 512], pb[:])

            # ---- main loop: load mask tile, subtract bias, store ----
            for b in range(B):
                for h in range(NH):
                    mt = io.tile([RPC, L], F32, tag="mask")
                    nc.sync.dma_start(out=mt[:], in_=mask_sh.ap()[b, h])
                    nc.vector.tensor_sub(
                        mt[:], mt[:], bias_sb[:, h * L : (h + 1) * L]
                    )
                    nc.scalar.dma_start(out=out_sh.ap()[b, h], in_=mt[:])

    nc.compile()
    return nc


def kernel(**inputs):
    global LAST_RESULTS
    mask = np.ascontiguousarray(np.asarray(inputs["mask"], dtype=np.float32))
    W0 = np.ascontiguousarray(np.asarray(inputs["W0"], dtype=np.float32))
    W1 = np.ascontiguousarray(np.asarray(inputs["W1"], dtype=np.float32))
    W2 = np.ascontiguousarray(np.asarray(inputs["W2"], dtype=np.float32))
    Wf = np.ascontiguousarray(np.asarray(inputs["Wf"], dtype=np.float32))
    b0 = np.asarray(inputs["b0"], dtype=np.float32).reshape(DIM, 1)
    b1 = np.asarray(inputs["b1"], dtype=np.float32).reshape(DIM, 1)
    b2 = np.asarray(inputs["b2"], dtype=np.float32).reshape(DIM, 1)
    bf = np.asarray(inputs["bf"], dtype=np.float32).reshape(NH, 1)
    jrev = np.ascontiguousarray(np.eye(128, dtype=np.float32)[::-1])

    if "prog" not in _CACHE:
        _CACHE["prog"] = _build_program()
    nc = _CACHE["prog"]

    s = np.arange(T, dtype=np.float32)
    in_maps = []
    for c in range(NCORES):
        r0 = c * RPC
        relrow = ((r0 + 127.0 - s) / DIVIDE).reshape(1, T).astype(np.float32)
        in_maps.append(
            {
                "mask_shard": np.ascontiguousarray(mask[:, :, r0 : r0 + RPC, :]),
                "relrow": relrow,
                "w0": W0,
                "w1": W1,
                "w2": W2,
                "wf": Wf,
                "b0c": b0,
                "b1c": b1,
                "b2c": b2,
                "bfc": bf,
                "jrev": jrev,
            }
        )

    trace = bool(int(os.environ.get("BASS_KERNEL_TRACE", "0")))
    res = run_bass_kernel_spmd(
        nc, in_maps, core_ids=list(range(NCORES)), trace=trace
    )
    LAST_RESULTS = res
    out = np.concatenate(
        [res.results[c]["out_shard"] for c in range(NCORES)], axis=2
    )
    return np.ascontiguousarray(out)


# revision 13
# speedup vs baseline: 1.7038x; 1.7038x over previous
"""Trainium2 Bass kernel for nn_DPB_24026047054525 (dense_mlp, memory-bound).

Computes: out[b,h,i,j] = mask[b,h,i,j] - bias[h,i,j] where
bias[h,i,j] = MLP((i-j)/DIVIDE)[h], MLP = 3x(Linear+SiLU) + Linear, widths
1->64->64->64->8.

Key structural insight: rel = (i-j) takes only 2047 distinct values, so the
MLP runs on a tiny per-core table of 1152 scalars instead of the 1M-point
grid.  The (128-row, 1024-col) per-core bias tile is a Toeplitz expansion of
that table:

    bias[p, j] = T[127 - p + j],   T[s] = MLP((r0 + 127 - s)/DIVIDE)

materialized on-chip as (a) a positive-stride overlapping "Hankel" DMA
H[k, h, t] = T[h, k + t] from a DRAM scratch copy of the table, and (b) a
partition-order reversal via an fp32 PE matmul against the anti-identity J
(out[p] = H[127-p], exact).  The subtract consumes the reversed chunks
directly from PSUM (no SBUF bias materialization).  All DMAs are vanilla
(partition-dim-first, positive strides).

Sharding: L1 (rows of the rel grid) split across 8 cores, 128 rows each;
mask/out sharded the same way; MLP weights replicated; subtract local.
q/k/v are unused by the reference computation and are never transferred.
"""

import os
import sys

import numpy as np

for _p in ("/opt/trn_rl_repo",):
    if _p not in sys.path and os.path.isdir(_p):
        sys.path.append(_p)

import concourse.bacc as bacc
import concourse.mybir as mybir
import concourse.tile as tile
from concourse.ap import AP
from concourse.bass_utils import run_bass_kernel_spmd

B = 2
NH = 8
L = 1024
DIM = 64
NCORES = 8
RPC = L // NCORES  # rows per core = 128
T = 1152  # per-core rel-table length (covers s in [0, 127+1023])
DIVIDE = 1.0

F32 = mybir.dt.float32

_CACHE = {}

# Filled by kernel() on every call with the BassKernelResults of the run.
LAST_RESULTS = None


def _build_program():
    nc = bacc.Bacc("TRN2", target_bir_lowering=False, debug=False)

    mask_sh = nc.dram_tensor("mask_shard", [B, NH, RPC, L], F32, kind="ExternalInput")
    # packA layout (64 x 204): [0:64]=W1, [64:128]=W2, [128:136]=Wf,
    # col 136=b0, 137=b1, 138=b2, 139 rows0:8=bf, row 0 cols [140:204]=W0.
    packa = nc.dram_tensor("packa", [DIM, 204], F32, kind="ExternalInput")
    relrow = nc.dram_tensor("relrow", [1, T], F32, kind="ExternalInput")
    jrev = nc.dram_tensor("jrev", [128, 128], F32, kind="ExternalInput")
    out_sh = nc.dram_tensor("out_shard", [B, NH, RPC, L], F32, kind="ExternalOutput")

    tbl_dram = nc.dram_tensor("tbl_scratch", [NH, T], F32)

    silu = mybir.ActivationFunctionType.Silu
    ident = mybir.ActivationFunctionType.Identity

    with tile.TileContext(nc) as tc:
        with (
            tc.tile_pool(name="consts", bufs=1) as consts,
            tc.tile_pool(name="hank", bufs=1) as hankp,
            tc.tile_pool(name="io", bufs=16) as io,
        ):
            # ---- constants to SBUF: 3 HWDGE DMAs ----
            pka = consts.tile([DIM, 204], F32)
            nc.sync.dma_start(out=pka[:], in_=packa.ap())
            rels = consts.tile([1, T], F32)
            nc.sync.dma_start(out=rels[:], in_=relrow.ap())
            jrevs = consts.tile([128, 128], F32)
            nc.sync.dma_start(out=jrevs[:], in_=jrev.ap())

            w1s = pka[:, 0:64]
            w2s = pka[:, 64:128]
            wfs = pka[:, 128:136]
            b0s = pka[:, 136:137]
            b1s = pka[:, 137:138]
            b2s = pka[:, 138:139]
            bfs = pka[0:8, 139:140]
            w0s = pka[0:1, 140:204]

            # ---- prefetch every mask tile (overlaps the MLP latency) ----
            mts = {}
            for h in range(NH):
                for b in range(B):
                    mt = io.tile([RPC, L], F32, tag="mask", name=f"mask_{h}_{b}")
                    nc.sync.dma_start(out=mt[:], in_=mask_sh.ap()[b, h])
                    mts[(h, b)] = mt

            # ---- MLP over the T-token rel table (transposed layout) ----
            with (
                tc.tile_pool(name="mlp", bufs=3) as mlp,
                tc.tile_pool(name="mlp_ps", bufs=3, space="PSUM") as mlp_ps,
            ):
                # Layer-major emission: engines run their streams in order,
                # so interleaving chunks lets PE run matmuls back-to-back
                # (ramping to the warm p-state) while ACT chases.
                gsb = mlp.tile([NH, T], F32, tag="gtable")
                chunks = ((0, 512), (512, 1024), (1024, T))
                layers = (
                    (w0s, b0s, silu),
                    (w1s, b1s, silu),
                    (w2s, b2s, silu),
                    (wfs, bfs, ident),
                )
                cur = {ci: rels[:, lo:hi] for ci, (lo, hi) in enumerate(chunks)}
                for li, (w, bcol, act) in enumerate(layers):
                    ps = {}
                    for ci, (lo, hi) in enumerate(chunks):
                        m = NH if li == 3 else DIM
                        ps[ci] = mlp_ps.tile([m, hi - lo], F32, tag=f"ps{li % 2}", name=f"ps_{li}_{ci}")
                        nc.tensor.matmul(
                            ps[ci][:], lhsT=w, rhs=cur[ci], start=True, stop=True
                        )
                    nxt = {}
                    for ci, (lo, hi) in enumerate(chunks):
                        if li == 3:
                            nc.scalar.activation(
                                gsb[:, lo:hi], ps[ci][:], act, bias=bcol, scale=1.0
                            )
                        else:
                            ht = mlp.tile([DIM, hi - lo], F32, tag=f"h{li}")
                            nc.scalar.activation(
                                ht[:], ps[ci][:], act, bias=bcol, scale=1.0
                            )
                            nxt[ci] = ht[:]
                    cur = nxt

                # ---- table to DRAM scratch ----
                nc.gpsimd.dma_start(out=tbl_dram.ap(), in_=gsb[:])
            tbl_h = tbl_dram.ap().tensor

            # ---- per-head pipeline ----
            with tc.tile_pool(name="rev_ps", bufs=6, space="PSUM") as rev_ps:
                for h in range(NH):
                    # Hankel slab H[k, t] = T[h, k + t]
                    hank = hankp.tile([128, L], F32, tag=f"hank{h}", name=f"hank_{h}")
                    src = AP(tbl_h, h * T, [[1, 128], [1, L]])
                    nc.gpsimd.dma_start(out=hank[:], in_=src)

                    # reversal chunks + subtract straight from PSUM
                    for c in range(2):
                        sl = slice(c * 512, (c + 1) * 512)
                        pb = rev_ps.tile([128, 512], F32, tag="rev", name=f"rev_{h}_{c}")
                        nc.tensor.matmul(
                            pb[:], lhsT=jrevs[:], rhs=hank[:, sl],
                            start=True, stop=True,
                        )
                        for b in range(B):
                            nc.vector.tensor_sub(
                                mts[(h, b)][:, sl], mts[(h, b)][:, sl], pb[:]
                            )

                    for b in range(B):
                        nc.scalar.dma_start(out=out_sh.ap()[b, h], in_=mts[(h, b)][:])

    nc.compile()
    return nc


def _pack_consts(W0, W1, W2, Wf, b0, b1, b2, bf):
    pka = np.zeros((DIM, 204), np.float32)
    pka[:, 0:64] = W1
    pka[:, 64:128] = W2
    pka[:, 128:136] = Wf
    pka[:, 136] = b0
    pka[:, 137] = b1
    pka[:, 138] = b2
    pka[0:8, 139] = bf
    pka[0, 140:204] = W0[0]
    return pka


def kernel(**inputs):
    global LAST_RESULTS
    mask = np.ascontiguousarray(np.asarray(inputs["mask"], dtype=np.float32))
    pka = _pack_consts(
        np.asarray(inputs["W0"], dtype=np.float32),
        np.asarray(inputs["W1"], dtype=np.float32),
        np.asarray(inputs["W2"], dtype=np.float32),
        np.asarray(inputs["Wf"], dtype=np.float32),
        np.asarray(inputs["b0"], dtype=np.float32),
        np.asarray(inputs["b1"], dtype=np.float32),
        np.asarray(inputs["b2"], dtype=np.float32),
        np.asarray(inputs["bf"], dtype=np.float32),
    )
    jrev = np.ascontiguousarray(np.eye(128, dtype=np.float32)[::-1])

    if "prog" not in _CACHE:
        _CACHE["prog"] = _build_program()
    nc = _CACHE["prog"]

    s = np.arange(T, dtype=np.float32)
    in_maps = []
    for c in range(NCORES):
        r0 = c * RPC
        relrow = ((r0 + 127.0 - s) / DIVIDE).reshape(1, T).astype(np.float32)
        in_maps.append(
            {
                "mask_shard": np.ascontiguousarray(mask[:, :, r0 : r0 + RPC, :]),
                "packa": pka,
                "relrow": relrow,
                "jrev": jrev,
            }
        )

    trace = bool(int(os.environ.get("BASS_KERNEL_TRACE", "0")))
    res = run_bass_kernel_spmd(
        nc, in_maps, core_ids=list(range(NCORES)), trace=trace
    )
    LAST_RESULTS = res
    out = np.concatenate(
        [res.results[c]["out_shard"] for c in range(NCORES)], axis=2
    )
    return np.ascontiguousarray(out)


# revision 20
# speedup vs baseline: 67875.3097x; 39836.8909x over previous
"""Trainium2 Bass kernel for nn_DPB_24026047054525 (dense_mlp, memory-bound).

Computes: out[b,h,i,j] = mask[b,h,i,j] - bias[h,i,j] where
bias[h,i,j] = MLP((i-j)/DIVIDE)[h], MLP = 3x(Linear+SiLU) + Linear, widths
1->64->64->64->8.

Key structural insight: rel = (i-j) takes only 2047 distinct values, so the
MLP runs on a tiny per-core table of 1152 scalars instead of the 1M-point
grid.  The (128-row, 1024-col) per-core bias tile is a Toeplitz expansion
of that table.  Feeding the MLP the rel values in ASCENDING order makes the
on-chip table the reversed one:

    Trev[s] = MLP((r0 - 1024 + s)/DIVIDE)            (s in [0, 1152))
    bias[p, j] = MLP((r0 + p - j)/DIVIDE) = Trev[1 + p + (1023 - j)]

so a vanilla positive-stride overlapping "Hankel" DMA M[p, t] = Trev[1+p+t]
from a DRAM scratch copy of the table, read back by the DVE with the free
dim reversed (in1 = M[:, ::-1], a legal negative free stride for compute
engines), yields the bias tile directly — no PE reversal, no PSUM staging.

Sharding: L1 (rows of the rel grid) split across 8 cores, 128 rows each;
mask/out sharded the same way; MLP weights replicated; subtract local.
q/k/v are unused by the reference computation and are never transferred.
"""

import os
import sys

import numpy as np

for _p in ("/opt/trn_rl_repo",):
    if _p not in sys.path and os.path.isdir(_p):
        sys.path.append(_p)

import concourse.bacc as bacc
import concourse.mybir as mybir
import concourse.tile as tile
from concourse.ap import AP
from concourse.bass_utils import run_bass_kernel_spmd

B = 2
NH = 8
L = 1024
DIM = 64
NCORES = 8
RPC = L // NCORES  # rows per core = 128
T = 1152  # per-core rel-table length
DIVIDE = 1.0

F32 = mybir.dt.float32

_CACHE = {}

# Filled by kernel() on every call with the BassKernelResults of the run.
LAST_RESULTS = None


def _build_program(reps=1):
    nc = bacc.Bacc("TRN2", target_bir_lowering=False, debug=False)

    mask_sh = nc.dram_tensor("mask_shard", [B, NH, RPC, L], F32, kind="ExternalInput")
    # packA layout (64 x 204): [0:64]=W1, [64:128]=W2, [128:136]=Wf,
    # col 136=b0, 137=b1, 138=b2, 139 rows0:8=bf, row 0 cols [140:204]=W0.
    packa = nc.dram_tensor("packa", [DIM, 204], F32, kind="ExternalInput")
    relrow = nc.dram_tensor("relrow", [1, T], F32, kind="ExternalInput")
    out_sh = nc.dram_tensor("out_shard", [B, NH, RPC, L], F32, kind="ExternalOutput")

    tbl_dram = nc.dram_tensor("tbl_scratch", [NH, T], F32)

    silu = mybir.ActivationFunctionType.Silu
    ident = mybir.ActivationFunctionType.Identity

    import contextlib

    with tile.TileContext(nc) as tc:
        with (
            tc.tile_pool(name="consts", bufs=1) as consts,
            tc.tile_pool(name="hank", bufs=1) as hankp,
            tc.tile_pool(name="io", bufs=16) as io,
            tc.For_i(0, reps, 1) if reps > 1 else contextlib.nullcontext(),
        ):
            # ---- constants to SBUF: 2 HWDGE DMAs ----
            pka = consts.tile([DIM, 204], F32)
            nc.sync.dma_start(out=pka[:], in_=packa.ap())
            rels = consts.tile([1, T], F32)
            nc.sync.dma_start(out=rels[:], in_=relrow.ap())

            w1s = pka[:, 0:64]
            w2s = pka[:, 64:128]
            wfs = pka[:, 128:136]
            b0s = pka[:, 136:137]
            b1s = pka[:, 137:138]
            b2s = pka[:, 138:139]
            bfs = pka[0:8, 139:140]
            w0s = pka[0:1, 140:204]

            # ---- prefetch every mask tile (overlaps the MLP latency) ----
            mts = {}
            for h in range(NH):
                for b in range(B):
                    mt = io.tile([RPC, L], F32, tag="mask", name=f"mask_{h}_{b}")
                    nc.sync.dma_start(out=mt[:], in_=mask_sh.ap()[b, h])
                    mts[(h, b)] = mt

            # ---- MLP over the T-token rel table (transposed layout) ----
            with (
                tc.tile_pool(name="mlp", bufs=3) as mlp,
                tc.tile_pool(name="mlp_ps", bufs=3, space="PSUM") as mlp_ps,
            ):
                # Layer-major emission: engines run their streams in order,
                # so interleaving chunks lets PE run matmuls back-to-back
                # (ramping toward the warm p-state) while ACT chases.
                gsb = mlp.tile([NH, T], F32, tag="gtable")
                chunks = ((0, 512), (512, 1024), (1024, T))
                layers = (
                    (w0s, b0s, silu),
                    (w1s, b1s, silu),
                    (w2s, b2s, silu),
                    (wfs, bfs, ident),
                )
                cur = {ci: rels[:, lo:hi] for ci, (lo, hi) in enumerate(chunks)}
                for li, (w, bcol, act) in enumerate(layers):
                    ps = {}
                    for ci, (lo, hi) in enumerate(chunks):
                        m = NH if li == 3 else DIM
                        ps[ci] = mlp_ps.tile(
                            [m, hi - lo], F32, tag=f"ps{li % 2}", name=f"ps_{li}_{ci}"
                        )
                        nc.tensor.matmul(
                            ps[ci][:], lhsT=w, rhs=cur[ci], start=True, stop=True
                        )
                    nxt = {}
                    for ci, (lo, hi) in enumerate(chunks):
                        if li == 3:
                            nc.scalar.activation(
                                gsb[:, lo:hi], ps[ci][:], act, bias=bcol, scale=1.0
                            )
                        else:
                            ht = mlp.tile(
                                [DIM, hi - lo], F32, tag=f"h{li}", name=f"h_{li}_{ci}"
                            )
                            nc.scalar.activation(
                                ht[:], ps[ci][:], act, bias=bcol, scale=1.0
                            )
                            nxt[ci] = ht[:]
                    cur = nxt

                # ---- table to DRAM scratch ----
                nc.gpsimd.dma_start(out=tbl_dram.ap(), in_=gsb[:])
            tbl_h = tbl_dram.ap().tensor

            # ---- per-head pipeline: Hankel slab + reversed-stride subtract ----
            for h in range(NH):
                # M[p, t] = Trev[h, 1 + p + t]  (overlapping, positive strides)
                hank = hankp.tile([128, L], F32, tag=f"hank{h}", name=f"hank_{h}")
                src = AP(tbl_h, h * T + 1, [[1, 128], [1, L]])
                nc.gpsimd.dma_start(out=hank[:], in_=src)

                # bias[p, j] = M[p, 1023 - j]: subtract with in1 free-reversed
                for b in range(B):
                    nc.vector.tensor_sub(
                        mts[(h, b)][:], mts[(h, b)][:], hank[:, ::-1]
                    )
                    nc.scalar.dma_start(out=out_sh.ap()[b, h], in_=mts[(h, b)][:])

    nc.compile()
    return nc


def _pack_consts(W0, W1, W2, Wf, b0, b1, b2, bf):
    pka = np.zeros((DIM, 204), np.float32)
    pka[:, 0:64] = W1
    pka[:, 64:128] = W2
    pka[:, 128:136] = Wf
    pka[:, 136] = b0
    pka[:, 137] = b1
    pka[:, 138] = b2
    pka[0:8, 139] = bf
    pka[0, 140:204] = W0[0]
    return pka


def _relrow_for_core(c):
    s = np.arange(T, dtype=np.float32)
    r0 = c * RPC
    return ((r0 - 1024.0 + s) / DIVIDE).reshape(1, T).astype(np.float32)


def kernel(**inputs):
    global LAST_RESULTS
    mask = np.ascontiguousarray(np.asarray(inputs["mask"], dtype=np.float32))
    pka = _pack_consts(
        np.asarray(inputs["W0"], dtype=np.float32),
        np.asarray(inputs["W1"], dtype=np.float32),
        np.asarray(inputs["W2"], dtype=np.float32),
        np.asarray(inputs["Wf"], dtype=np.float32),
        np.asarray(inputs["b0"], dtype=np.float32),
        np.asarray(inputs["b1"], dtype=np.float32),
        np.asarray(inputs["b2"], dtype=np.float32),
        np.asarray(inputs["bf"], dtype=np.float32),
    )

    if "prog" not in _CACHE:
        _CACHE["prog"] = _build_program()
    nc = _CACHE["prog"]

    in_maps = []
    for c in range(NCORES):
        r0 = c * RPC
        in_maps.append(
            {
                "mask_shard": np.ascontiguousarray(mask[:, :, r0 : r0 + RPC, :]),
                "packa": pka,
                "relrow": _relrow_for_core(c),
            }
        )

    trace = bool(int(os.environ.get("BASS_KERNEL_TRACE", "0")))
    res = run_bass_kernel_spmd(
        nc, in_maps, core_ids=list(range(NCORES)), trace=trace
    )
    LAST_RESULTS = res
    out = np.concatenate(
        [res.results[c]["out_shard"] for c in range(NCORES)], axis=2
    )
    return np.ascontiguousarray(out)
